# revision 17
# baseline (speedup 1.0000x reference)
"""Trainium2 Bass kernel for nn_ClippedReLU (piecewise-linear clip).

Reference semantics:
    eta = eta_fault[Mask]                 # [B, F, 4] rows (y0, y1, x0, x1)
    s   = (y1-y0)/(x1-x0)
    lin = y0 + s*(z - x0)
    out = where(z < x0, y0, where(z <= x1, lin, y1))

For rows with x1 > x0 (all rows of the standard table) this equals
    out = min(max(y0 + s*(z-x0), min(y0,y1)), max(y0,y1))
computed with the exact same f32 op order as the reference, so results are
bitwise identical. The per-(b,f) params are tiny and derived on the host;
the device streams z (256 MiB in / 256 MiB out -> memory-bound).

Sharding: data-parallel across 8 cores; core i takes b = i//2 and N-half
i%2 (a contiguous [8, 1024, 1024] block), so each core sees a single b and
one param vector per f.

Device pipeline per row-supertile [512 rows x 1024 f] of the [8192, 1024]
shard (params vary per f, so tensor ops want f on partitions):
  1. 2 MiB DMA in (SP HWDGE ring)          z_tile [128, 4, 1024]
  2. PE transposes [128,128] blocks        -> psum [f=128, rows=512]
  3. DVE tensor_scalar  d = (z' - x0[p]) * s[p]     (PSUM -> SBUF)
  4. ACT activation     e = d + y0[p]               (Identity, bias AP)
  5. GPSIMD tensor_scalar o = min(max(e, lo[p]), hi[p])
  6. PE transposes back                    -> psum [rows=128, f-block]
  7. ACT/DVE copies psum -> SBUF out tile
  8. 2 MiB DMA out (ACT HWDGE ring, so prefetch DMAs are not blocked
     behind compute-gated stores in the SP FIFO)
Every engine stays below the ~187 us/core DMA roofline (64 MiB @ ~360 GB/s).

Degenerate rows (x1 <= x0 or non-finite slope; impossible with the standard
table) are patched on the host with exact reference semantics afterwards.
"""

import numpy as np

import concourse.bacc as bacc
import concourse.mybir as mybir
from concourse.tile import TileContext
from concourse.bass_utils import run_bass_kernel_spmd

B, N, M, F = 4, 16, 1024, 1024
NCORES = 8
NH = N // 2                # N-rows per core
ROWS = NH * M              # 8192 flattened rows per core
P = 128                    # SBUF partitions
SR = 512                   # supertile rows
SF = 512                   # compute-tile f width
RC = SR // P               # 4 row chunks per supertile
FB = SF // P               # 4 f-blocks per compute tile
NG = F // P                # 8 global f-blocks
NST_R = ROWS // SR         # 16 row-supertiles
NST_F = F // SF            # 2 compute tiles per row-supertile

_nc_cache = {}


def _build_nc():
    f32 = mybir.dt.float32
    nc = bacc.Bacc("TRN2", debug=False)
    z = nc.dram_tensor("z", [ROWS, F], f32, kind="ExternalInput")
    params = nc.dram_tensor("params", [P, 5, NG], f32, kind="ExternalInput")
    eye = nc.dram_tensor("eye", [P, P], f32, kind="ExternalInput")
    out = nc.dram_tensor("out", [ROWS, F], f32, kind="ExternalOutput")

    # [t, p, rc, f]: row = (t*RC + rc)*P + p
    zt = z.rearrange("(t rc p) f -> t p rc f", rc=RC, p=P)
    ot = out.rearrange("(t rc p) f -> t p rc f", rc=RC, p=P)

    sub = mybir.AluOpType.subtract
    mul = mybir.AluOpType.mult
    amax = mybir.AluOpType.max
    amin = mybir.AluOpType.min

    with TileContext(nc) as tc:
        with (
            tc.tile_pool(name="pp", bufs=1) as pp,
            tc.tile_pool(name="io", bufs=3) as io,
            tc.tile_pool(name="sb", bufs=4) as sbp,
            tc.tile_pool(name="pin", bufs=3, space="PSUM") as pin,
            tc.tile_pool(name="pout", bufs=5, space="PSUM") as pout,
        ):
            pt = pp.tile([P, 5, NG], f32, tag="params")
            nc.sync.dma_start(out=pt, in_=params[:, :, :])
            eyet = pp.tile([P, P], f32, tag="eye")
            nc.sync.dma_start(out=eyet, in_=eye[:, :])

            for tr in range(NST_R):
                zt_t = io.tile([P, RC, F], f32, tag="z")
                nc.sync.dma_start(out=zt_t, in_=zt[tr])
                outt = io.tile([P, RC, F], f32, tag="o")
                for tf in range(NST_F):
                    pouts = [
                        pout.tile([P, SF], f32, tag="pout", name=f"po_{tr}_{tf}_{rc}")
                        for rc in range(RC)
                    ]
                    for fb in range(FB):
                        g = tf * FB + fb
                        pin_t = pin.tile([P, SR], f32, tag="pin")
                        for rc in range(RC):
                            nc.tensor.transpose(
                                pin_t[:, rc * P:(rc + 1) * P],
                                zt_t[:, rc, g * P:(g + 1) * P],
                                eyet,
                            )
                        sb1 = sbp.tile([P, SR], f32, tag="sb1")
                        nc.vector.tensor_scalar(
                            sb1, pin_t, pt[:, 1, g:g + 1], pt[:, 0, g:g + 1], sub, mul
                        )
                        sb2 = sbp.tile([P, SR], f32, tag="sb2")
                        nc.scalar.activation(
                            sb2, sb1, mybir.ActivationFunctionType.Identity,
                            bias=pt[:, 2, g:g + 1], scale=1.0,
                        )
                        sb3 = sbp.tile([P, SR], f32, tag="sb3")
                        nc.gpsimd.tensor_scalar(
                            sb3, sb2, pt[:, 3, g:g + 1], pt[:, 4, g:g + 1], amax, amin
                        )
                        for rc in range(RC):
                            nc.tensor.transpose(
                                pouts[rc][:, fb * P:(fb + 1) * P],
                                sb3[:, rc * P:(rc + 1) * P],
                                eyet,
                            )
                    for rc in range(RC):
                        dst = outt[:, rc, tf * SF:(tf + 1) * SF]
                        if rc % 2 == 0:
                            nc.scalar.copy(dst, pouts[rc])
                        else:
                            nc.vector.tensor_copy(dst, pouts[rc])
                nc.scalar.dma_start(out=ot[tr], in_=outt)
    nc.compile()
    return nc


def _host_params(eta_np):
    """Per-row params (f32, reference rounding). Returns (s, x0, y0, lo, hi, bad)."""
    eta_np = eta_np.astype(np.float32)
    y0 = eta_np[:, 0]
    y1 = eta_np[:, 1]
    x0 = eta_np[:, 2]
    x1 = eta_np[:, 3]
    dx = x1 - x0                                   # f32, as in reference
    with np.errstate(divide="ignore", invalid="ignore"):
        s = (y1 - y0) / dx                         # f32, bitwise matches XLA
    lo = np.minimum(y0, y1)
    hi = np.maximum(y0, y1)
    # clamp(y0 + s*(z-x0), lo, hi) == reference only when x1 > x0, s finite
    bad = ~((dx > 0) & np.isfinite(s))
    z32 = np.float32(0)
    return (np.where(bad, z32, s), np.where(bad, z32, x0),
            np.where(bad, z32, y0), np.where(bad, z32, lo),
            np.where(bad, z32, hi), bad)


def _param_pack(s, x0, y0, lo, hi):
    """[F] arrays -> [P, 5, NG] with element (p, j, g) = param_j[g*P + p]."""
    stack = np.stack([s, x0, y0, lo, hi])            # [5, F]
    return np.ascontiguousarray(
        stack.reshape(5, NG, P).transpose(2, 0, 1)   # [P, 5, NG]
    )


def make_in_maps(z, Mask, eta):
    """Shard z over cores and build per-core input maps. Returns (in_maps, bad_bf)."""
    s_r, x0_r, y0_r, lo_r, hi_r, bad_r = _host_params(eta)
    mask_i = Mask.astype(np.int64)
    par_bf = [a[mask_i] for a in (s_r, x0_r, y0_r, lo_r, hi_r)]   # each [B, F]
    bad_bf = bad_r[mask_i]
    eye = np.eye(P, dtype=np.float32)

    in_maps = []
    for core in range(NCORES):
        b, nh = core // 2, core % 2
        zs = z[b, nh * NH:(nh + 1) * NH].reshape(ROWS, F)
        in_maps.append({
            "z": zs,
            "params": _param_pack(*[a[b] for a in par_bf]),
            "eye": eye,
        })
    return in_maps, bad_bf


def kernel(z, Mask, eta_fault):
    z = np.ascontiguousarray(np.asarray(z, dtype=np.float32))
    Mask = np.asarray(Mask)
    eta = np.asarray(eta_fault, dtype=np.float32)

    if "nc" not in _nc_cache:
        _nc_cache["nc"] = _build_nc()
    nc = _nc_cache["nc"]

    in_maps, bad_bf = make_in_maps(z, Mask, eta)
    mask_i = Mask.astype(np.int64)

    res = run_bass_kernel_spmd(nc, in_maps, list(range(NCORES)))

    out = np.empty((B, N, M, F), dtype=np.float32)
    for core in range(NCORES):
        b, nh = core // 2, core % 2
        out[b, nh * NH:(nh + 1) * NH] = res.results[core]["out"].reshape(NH, M, F)

    # Host patch for degenerate rows (never triggers with the standard table).
    if bad_bf.any():
        eta_g = eta[mask_i]  # [B, F, 4] f32
        for b in range(B):
            (fbad,) = np.nonzero(bad_bf[b])
            if fbad.size == 0:
                continue
            y0 = eta_g[b, fbad, 0]
            y1 = eta_g[b, fbad, 1]
            x0 = eta_g[b, fbad, 2]
            x1 = eta_g[b, fbad, 3]
            zb = z[b][:, :, fbad]
            with np.errstate(divide="ignore", invalid="ignore"):
                lin = y0 + (y1 - y0) / (x1 - x0) * (zb - x0)
            out[b][:, :, fbad] = np.where(
                zb < x0, y0, np.where(zb <= x1, lin, y1)
            ).astype(np.float32)

    return out


# revision 19
# speedup vs baseline: 5.0102x; 5.0102x over previous
"""Trainium2 Bass kernel for nn_ClippedReLU (piecewise-linear clip).

Reference semantics:
    eta = eta_fault[Mask]                 # [B, F, 4] rows (y0, y1, x0, x1)
    s   = (y1-y0)/(x1-x0)
    lin = y0 + s*(z - x0)
    out = where(z < x0, y0, where(z <= x1, lin, y1))

For rows with x1 > x0 (all rows of the standard table) this equals
    out = min(max(y0 + s*(z-x0), min(y0,y1)), max(y0,y1))
computed with the exact same f32 op order as the reference, so results are
bitwise identical. The per-(b,f) params are tiny and derived on the host;
the device streams z (256 MiB in / 256 MiB out -> memory-bound).

Sharding: data-parallel across 8 cores; core i takes b = i//2 and N-half
i%2 (a contiguous [8, 1024, 1024] block), so each core sees a single b and
one param vector per f.

Device pipeline per row-supertile [512 rows x 1024 f] of the [8192, 1024]
shard (params vary per f, so tensor ops want f on partitions):
  1. 2 MiB DMA in (SP HWDGE ring)          z_tile [128, 4, 1024]
  2. PE transposes [128,128] blocks        -> psum [f=128, rows=512]
  3. DVE tensor_scalar  d = (z' - x0[p]) * s[p]     (PSUM -> SBUF)
  4. ACT activation     e = d + y0[p]               (Identity, bias AP)
  5. GPSIMD tensor_scalar o = min(max(e, lo[p]), hi[p])
  6. PE transposes back                    -> psum [rows=128, f-block]
  7. ACT/DVE copies psum -> SBUF out tile
  8. 2 MiB DMA out (ACT HWDGE ring, so prefetch DMAs are not blocked
     behind compute-gated stores in the SP FIFO)
Every engine stays below the ~187 us/core DMA roofline (64 MiB @ ~360 GB/s).

Degenerate rows (x1 <= x0 or non-finite slope; impossible with the standard
table) are patched on the host with exact reference semantics afterwards.
"""

import numpy as np

import concourse.bacc as bacc
import concourse.mybir as mybir
from concourse.tile import TileContext
from concourse.bass_utils import run_bass_kernel_spmd

B, N, M, F = 4, 16, 1024, 1024
NCORES = 8
NH = N // 2                # N-rows per core
ROWS = NH * M              # 8192 flattened rows per core
P = 128                    # SBUF partitions
SR = 512                   # supertile rows
SF = 512                   # compute-tile f width
RC = SR // P               # 4 row chunks per supertile
FB = SF // P               # 4 f-blocks per compute tile
NG = F // P                # 8 global f-blocks
NST_R = ROWS // SR         # 16 row-supertiles
NST_F = F // SF            # 2 compute tiles per row-supertile

_nc_cache = {}


def _build_nc():
    f32 = mybir.dt.float32
    nc = bacc.Bacc("TRN2", debug=False)
    z = nc.dram_tensor("z", [ROWS, F], f32, kind="ExternalInput")
    params = nc.dram_tensor("params", [P, 5, NG], f32, kind="ExternalInput")
    eye = nc.dram_tensor("eye", [P, P], f32, kind="ExternalInput")
    out = nc.dram_tensor("out", [ROWS, F], f32, kind="ExternalOutput")

    # [t, p, rc, f]: row = (t*RC + rc)*P + p
    zt = z.rearrange("(t rc p) f -> t p rc f", rc=RC, p=P)
    ot = out.rearrange("(t rc p) f -> t p rc f", rc=RC, p=P)

    sub = mybir.AluOpType.subtract
    mul = mybir.AluOpType.mult
    amax = mybir.AluOpType.max
    amin = mybir.AluOpType.min

    with TileContext(nc) as tc:
        with (
            tc.tile_pool(name="pp", bufs=1) as pp,
            tc.tile_pool(name="io", bufs=3) as io,
            tc.tile_pool(name="sb", bufs=4) as sbp,
            tc.tile_pool(name="pin", bufs=3, space="PSUM") as pin,
            tc.tile_pool(name="pout", bufs=5, space="PSUM") as pout,
        ):
            pt = pp.tile([P, 5, NG], f32, tag="params")
            nc.sync.dma_start(out=pt, in_=params[:, :, :])
            eyet = pp.tile([P, P], f32, tag="eye")
            nc.sync.dma_start(out=eyet, in_=eye[:, :])

            for tr in range(NST_R):
                zt_t = io.tile([P, RC, F], f32, tag="z")
                nc.sync.dma_start(out=zt_t, in_=zt[tr])
                outt = io.tile([P, RC, F], f32, tag="o")
                for tf in range(NST_F):
                    pouts = [
                        pout.tile([P, SF], f32, tag="pout", name=f"po_{tr}_{tf}_{rc}")
                        for rc in range(RC)
                    ]
                    for fb in range(FB):
                        g = tf * FB + fb
                        pin_t = pin.tile([P, SR], f32, tag="pin")
                        for rc in range(RC):
                            nc.tensor.transpose(
                                pin_t[:, rc * P:(rc + 1) * P],
                                zt_t[:, rc, g * P:(g + 1) * P],
                                eyet,
                            )
                        sb1 = sbp.tile([P, SR], f32, tag="sb1")
                        nc.vector.tensor_scalar(
                            sb1, pin_t, pt[:, 1, g:g + 1], pt[:, 0, g:g + 1], sub, mul
                        )
                        sb2 = sbp.tile([P, SR], f32, tag="sb2")
                        nc.scalar.activation(
                            sb2, sb1, mybir.ActivationFunctionType.Identity,
                            bias=pt[:, 2, g:g + 1], scale=1.0,
                        )
                        sb3 = sbp.tile([P, SR], f32, tag="sb3")
                        nc.vector.tensor_scalar(
                            sb3, sb2, pt[:, 3, g:g + 1], pt[:, 4, g:g + 1], amax, amin
                        )
                        for rc in range(RC):
                            nc.tensor.transpose(
                                pouts[rc][:, fb * P:(fb + 1) * P],
                                sb3[:, rc * P:(rc + 1) * P],
                                eyet,
                            )
                    for rc in range(RC):
                        dst = outt[:, rc, tf * SF:(tf + 1) * SF]
                        nc.scalar.copy(dst, pouts[rc])
                nc.scalar.dma_start(out=ot[tr], in_=outt)
    nc.compile()
    return nc


def _host_params(eta_np):
    """Per-row params (f32, reference rounding). Returns (s, x0, y0, lo, hi, bad)."""
    eta_np = eta_np.astype(np.float32)
    y0 = eta_np[:, 0]
    y1 = eta_np[:, 1]
    x0 = eta_np[:, 2]
    x1 = eta_np[:, 3]
    dx = x1 - x0                                   # f32, as in reference
    with np.errstate(divide="ignore", invalid="ignore"):
        s = (y1 - y0) / dx                         # f32, bitwise matches XLA
    lo = np.minimum(y0, y1)
    hi = np.maximum(y0, y1)
    # clamp(y0 + s*(z-x0), lo, hi) == reference only when x1 > x0, s finite
    bad = ~((dx > 0) & np.isfinite(s))
    z32 = np.float32(0)
    return (np.where(bad, z32, s), np.where(bad, z32, x0),
            np.where(bad, z32, y0), np.where(bad, z32, lo),
            np.where(bad, z32, hi), bad)


def _param_pack(s, x0, y0, lo, hi):
    """[F] arrays -> [P, 5, NG] with element (p, j, g) = param_j[g*P + p]."""
    stack = np.stack([s, x0, y0, lo, hi])            # [5, F]
    return np.ascontiguousarray(
        stack.reshape(5, NG, P).transpose(2, 0, 1)   # [P, 5, NG]
    )


def make_in_maps(z, Mask, eta):
    """Shard z over cores and build per-core input maps. Returns (in_maps, bad_bf)."""
    s_r, x0_r, y0_r, lo_r, hi_r, bad_r = _host_params(eta)
    mask_i = Mask.astype(np.int64)
    par_bf = [a[mask_i] for a in (s_r, x0_r, y0_r, lo_r, hi_r)]   # each [B, F]
    bad_bf = bad_r[mask_i]
    eye = np.eye(P, dtype=np.float32)

    in_maps = []
    for core in range(NCORES):
        b, nh = core // 2, core % 2
        zs = z[b, nh * NH:(nh + 1) * NH].reshape(ROWS, F)
        in_maps.append({
            "z": zs,
            "params": _param_pack(*[a[b] for a in par_bf]),
            "eye": eye,
        })
    return in_maps, bad_bf


def kernel(z, Mask, eta_fault):
    z = np.ascontiguousarray(np.asarray(z, dtype=np.float32))
    Mask = np.asarray(Mask)
    eta = np.asarray(eta_fault, dtype=np.float32)

    if "nc" not in _nc_cache:
        _nc_cache["nc"] = _build_nc()
    nc = _nc_cache["nc"]

    in_maps, bad_bf = make_in_maps(z, Mask, eta)
    mask_i = Mask.astype(np.int64)

    res = run_bass_kernel_spmd(nc, in_maps, list(range(NCORES)))

    out = np.empty((B, N, M, F), dtype=np.float32)
    for core in range(NCORES):
        b, nh = core // 2, core % 2
        out[b, nh * NH:(nh + 1) * NH] = res.results[core]["out"].reshape(NH, M, F)

    # Host patch for degenerate rows (never triggers with the standard table).
    if bad_bf.any():
        eta_g = eta[mask_i]  # [B, F, 4] f32
        for b in range(B):
            (fbad,) = np.nonzero(bad_bf[b])
            if fbad.size == 0:
                continue
            y0 = eta_g[b, fbad, 0]
            y1 = eta_g[b, fbad, 1]
            x0 = eta_g[b, fbad, 2]
            x1 = eta_g[b, fbad, 3]
            zb = z[b][:, :, fbad]
            with np.errstate(divide="ignore", invalid="ignore"):
                lin = y0 + (y1 - y0) / (x1 - x0) * (zb - x0)
            out[b][:, :, fbad] = np.where(
                zb < x0, y0, np.where(zb <= x1, lin, y1)
            ).astype(np.float32)

    return out
